# revision 25
# baseline (speedup 1.0000x reference)
"""Trainium2 Bass kernel for L1 + SSIM diffusion loss.

loss = mean|x-y| + 0.1 * (1 - mean(ssim_map(x, y)))

Data-parallel over 8 NeuronCores: each core processes 1024 images
(3072 channel-images of 32x32). Per core the SSIM separable gaussian
blurs are computed on the tensor engine as banded matmuls:
  H-blur:  out1 = M4^T @ tile      (block-diag banded lhsT, 4 row-blocks)
  32x32 block transpose on DVE
  W-blur:  out3 = W4^T @ out1^T'   (block-diag banded lhsT)
The SSIM algebra runs in the S=x+y / D=x-y basis:
  P = B(S) = mu1+mu2, Q = B(D) = mu1-mu2
  2*mu1*mu2   = (P^2-Q^2)/2        mu1^2+mu2^2 = (P^2+Q^2)/2
  2*sigma12   = (B(S^2)-B(D^2))/2 - (P^2-Q^2)/2
  sig1+sig2   = (B(S^2)+B(D^2))/2 - (P^2+Q^2)/2
Per-core partial sums (sum|D| and sum ssim_map) are returned as
[128, n_groups] stat tiles and combined on the host.
"""

import os
import sys

sys.path.insert(0, "/opt/trn_rl_repo")

import math
from contextlib import ExitStack

DBG_STAGE = int(os.environ.get("K_STAGE", "8"))

import numpy as np

import concourse.bass as bass
import concourse.tile as tile
from concourse import bacc, mybir
from concourse.bass_utils import run_bass_kernel_spmd

F32 = mybir.dt.float32

N_CORES = 8
BATCH = 8192
CH = 3
HW = 32
WIN = 11
OUT = HW - WIN + 1  # 22
SIGMA = 1.5
DATA_RANGE = 1.0
K1, K2 = 0.01, 0.03
C1 = (K1 * DATA_RANGE) ** 2
C2 = (K2 * DATA_RANGE) ** 2
SSIM_WEIGHT = 0.1

CHIMGS_PER_CORE = BATCH // N_CORES * CH  # 3072
GROUP = 64  # channel-images per group
N_GROUPS = CHIMGS_PER_CORE // GROUP  # 48


def _gaussian_1d():
    coords = np.arange(WIN, dtype=np.float64) - (WIN - 1) / 2.0
    g = np.exp(-(coords**2) / (2.0 * SIGMA**2))
    g = g / g.sum()
    return g.astype(np.float32)


def _blur_mats():
    """M: [OUT, HW] h-blur matrix (out = M @ img); same matrix for w-blur."""
    g = _gaussian_1d()
    M = np.zeros((OUT, HW), dtype=np.float32)
    for i in range(OUT):
        M[i, i : i + WIN] = g
    return M


def make_consts():
    M = _blur_mats()
    # H-blur lhsT: [128, 128] block-diag of 4x M^T (padded 22->32 out rows)
    m4t = np.zeros((128, 128), dtype=np.float32)
    for b in range(4):
        m4t[b * 32 : b * 32 + HW, b * 32 : b * 32 + OUT] = M.T
    # W-blur lhsT: [128, 88] block-diag of 4x M^T (dense 22 out cols)
    w4 = np.zeros((128, 88), dtype=np.float32)
    for b in range(4):
        w4[b * 32 : b * 32 + HW, b * 22 : b * 22 + OUT] = M.T
    w4h = (0.5 * w4).astype(np.float32)
    w4hn = (-0.5 * w4).astype(np.float32)
    return m4t, w4, w4h, w4hn


def build_kernel(n_groups=N_GROUPS):
    nc = bacc.Bacc(
        "TRN2", target_bir_lowering=False, debug=False, num_devices=N_CORES
    )
    n_chimgs = n_groups * GROUP
    x_ap = nc.dram_tensor(
        "x", [n_chimgs, HW * HW], F32, kind="ExternalInput"
    ).ap()
    y_ap = nc.dram_tensor(
        "y", [n_chimgs, HW * HW], F32, kind="ExternalInput"
    ).ap()
    m4t_ap = nc.dram_tensor("m4t", [128, 128], F32, kind="ExternalInput").ap()
    w4_ap = nc.dram_tensor("w4", [128, 88], F32, kind="ExternalInput").ap()
    w4h_ap = nc.dram_tensor("w4h", [128, 88], F32, kind="ExternalInput").ap()
    w4hn_ap = nc.dram_tensor("w4hn", [128, 88], F32, kind="ExternalInput").ap()
    l1_out = nc.dram_tensor(
        "l1stat", [128, n_groups], F32, kind="ExternalOutput"
    ).ap()
    ssim_out = nc.dram_tensor(
        "ssimstat", [128, n_groups], F32, kind="ExternalOutput"
    ).ap()

    with tile.TileContext(nc) as tc:
        with ExitStack() as ctx:
            kernel_body(
                ctx, tc, x_ap, y_ap, m4t_ap, w4_ap, w4h_ap, w4hn_ap,
                l1_out, ssim_out, n_groups,
            )
    nc.compile()
    return nc


def kernel_body(ctx, tc, x_ap, y_ap, m4t_ap, w4_ap, w4h_ap, w4hn_ap,
                l1_out, ssim_out, n_groups):
    nc = tc.nc
    sub = mybir.AluOpType.subtract
    add = mybir.AluOpType.add
    mult = mybir.AluOpType.mult
    SQ = mybir.ActivationFunctionType.Square
    ABS = mybir.ActivationFunctionType.Abs
    CPY = mybir.ActivationFunctionType.Copy

    consts = ctx.enter_context(tc.tile_pool(name="consts", bufs=1))
    inp = ctx.enter_context(tc.tile_pool(name="inp", bufs=3))
    maps = ctx.enter_context(tc.tile_pool(name="maps", bufs=2))
    psum1 = ctx.enter_context(tc.tile_pool(name="psum1", bufs=3, space="PSUM"))
    psum3 = ctx.enter_context(tc.tile_pool(name="psum3", bufs=1, space="PSUM"))
    tts = ctx.enter_context(tc.tile_pool(name="tts", bufs=2))
    alg = ctx.enter_context(tc.tile_pool(name="alg", bufs=2))
    stats = ctx.enter_context(tc.tile_pool(name="stats", bufs=1))

    m4t = consts.tile([128, 128], F32)
    nc.sync.dma_start(m4t[:], m4t_ap[:])
    w4 = consts.tile([128, 88], F32)
    nc.sync.dma_start(w4[:], w4_ap[:])
    w4h = consts.tile([128, 88], F32)
    nc.sync.dma_start(w4h[:], w4h_ap[:])
    w4hn = consts.tile([128, 88], F32)
    nc.sync.dma_start(w4hn[:], w4hn_ap[:])

    l1_stat = stats.tile([128, n_groups], F32, tag="l1stat")
    ssim_stat = stats.tile([128, n_groups], F32, tag="ssimstat")
    nc.vector.memset(l1_stat[:], 0.0)
    nc.vector.memset(ssim_stat[:], 0.0)
    c1b = consts.tile([128, 1], F32, tag="c1b")
    nc.vector.memset(c1b[:], C1)
    c2b = consts.tile([128, 1], F32, tag="c2b")
    nc.vector.memset(c2b[:], C2)

    for g in range(n_groups):
        # ---- load: [128, 512] = (b,k) x (q,j); chimg c = g*64 + b*16 + q
        x_t = inp.tile([128, 512], F32, tag="x")
        y_t = inp.tile([128, 512], F32, tag="y")
        for t, ap in ((x_t, x_ap), (y_t, y_ap)):
            for b in range(4):
                src = ap[
                    g * GROUP + b * 16 : g * GROUP + (b + 1) * 16, :
                ].rearrange("q (k j) -> k q j", k=HW, j=HW)
                dst = t[b * 32 : (b + 1) * 32, :].rearrange(
                    "k (q j) -> k q j", q=16, j=HW
                )
                nc.sync.dma_start(dst, src)

        def keep_live(t, col):
            j = maps.tile(list(t.shape), F32, tag="keeplive")
            nc.scalar.activation(
                j[: t.shape[0]], t[:], ABS,
                accum_out=ssim_stat[: t.shape[0], col : col + 1],
            )

        if DBG_STAGE == 1:
            keep_live(x_t, g)
            keep_live(y_t, g)
            continue

        # ---- S, D, S^2, D^2, |D| accumulation
        s_t = maps.tile([128, 512], F32, tag="S")
        nc.vector.tensor_add(s_t[:], x_t[:], y_t[:])
        d_t = maps.tile([128, 512], F32, tag="D")
        nc.vector.tensor_sub(d_t[:], x_t[:], y_t[:])
        s2_t = maps.tile([128, 512], F32, tag="S2")
        nc.scalar.activation(s2_t[:], s_t[:], SQ)
        d2_t = maps.tile([128, 512], F32, tag="D2")
        abs_junk = maps.tile([128, 512], F32, tag="absjunk")
        nc.scalar.activation(
            abs_junk[:], d_t[:], ABS, accum_out=l1_stat[:, g : g + 1]
        )
        nc.scalar.activation(d2_t[:], d_t[:], SQ)

        if DBG_STAGE == 2:
            keep_live(s2_t, g)
            keep_live(d2_t, g)
            continue

        # ---- H-blur + transpose per map
        tts_of = {}
        for name, src_t in (
            ("S", s_t), ("D", d_t), ("S2", s2_t), ("D2", d2_t),
        ):
            o1 = psum1.tile([128, 512], F32, tag="out1")
            nc.tensor.matmul(o1[:], m4t[:], src_t[:], start=True, stop=True)
            o1s = tts.tile([128, 512], F32, tag="o1s" + name)
            nc.scalar.copy(o1s[:], o1[:])
            tt = tts.tile([128, 512], F32, tag="tt" + name)
            nc.vector.transpose(tt[:], o1s[:])
            tts_of[name] = tt

        if DBG_STAGE == 3:
            for tt in tts_of.values():
                keep_live(tt, g)
            continue

        def wrhs(tt):
            return tt[:].rearrange("p (q i) -> p q i", q=16, i=HW)[:, :, 0:OUT]

        # ---- W-blur matmuls; G/H formed by PSUM accumulation
        P = psum3.tile([88, 352], F32, tag="out3P")
        nc.tensor.matmul(P[:], w4[:], wrhs(tts_of["S"]), start=True, stop=True)
        Q = psum3.tile([88, 352], F32, tag="out3Q")
        nc.tensor.matmul(Q[:], w4[:], wrhs(tts_of["D"]), start=True, stop=True)
        G = psum3.tile([88, 352], F32, tag="out3G")  # 2 B(xy)
        nc.tensor.matmul(G[:], w4h[:], wrhs(tts_of["S2"]), start=True, stop=False)
        nc.tensor.matmul(G[:], w4hn[:], wrhs(tts_of["D2"]), start=False, stop=True)
        H = psum3.tile([88, 352], F32, tag="out3H")  # B(x^2) + B(y^2)
        nc.tensor.matmul(H[:], w4h[:], wrhs(tts_of["S2"]), start=True, stop=False)
        nc.tensor.matmul(H[:], w4h[:], wrhs(tts_of["D2"]), start=False, stop=True)

        if DBG_STAGE == 4:
            for t in (P, Q, G, H):
                keep_live(t, g)
            continue

        # ---- ssim algebra on [88, 352]
        rt = math.sqrt(0.5)
        U = alg.tile([88, 352], F32, tag="U")
        nc.scalar.activation(U[:], P[:], SQ, scale=rt)  # 0.5*P^2
        V = alg.tile([88, 352], F32, tag="V")
        nc.scalar.activation(V[:], Q[:], SQ, scale=rt)  # 0.5*Q^2
        A1 = alg.tile([88, 352], F32, tag="A1")
        nc.vector.tensor_sub(A1[:], U[:], V[:])  # 2 mu1 mu2
        A2 = alg.tile([88, 352], F32, tag="A2")
        nc.vector.tensor_add(A2[:], U[:], V[:])  # mu1^2 + mu2^2
        num1 = alg.tile([88, 352], F32, tag="num1")
        nc.scalar.add(num1[:], A1[:], c1b[:88, :])
        den1 = alg.tile([88, 352], F32, tag="den1")
        nc.scalar.add(den1[:], A2[:], c1b[:88, :])
        tn = alg.tile([88, 352], F32, tag="tn")
        nc.vector.tensor_sub(tn[:], G[:], A1[:])  # 2 sigma12
        num2 = alg.tile([88, 352], F32, tag="num2")
        nc.scalar.add(num2[:], tn[:], c2b[:88, :])
        td = alg.tile([88, 352], F32, tag="td")
        nc.vector.tensor_sub(td[:], H[:], A2[:])  # sig1^2 + sig2^2
        den2 = alg.tile([88, 352], F32, tag="den2")
        nc.scalar.add(den2[:], td[:], c2b[:88, :])
        if DBG_STAGE == 5:
            for t in (num1, num2, den1, den2):
                keep_live(t, g)
            continue
        nn = alg.tile([88, 352], F32, tag="nn")
        nc.vector.tensor_mul(nn[:], num1[:], num2[:])
        dd = alg.tile([88, 352], F32, tag="dd")
        nc.vector.tensor_mul(dd[:], den1[:], den2[:])
        if DBG_STAGE == 6:
            keep_live(nn, g)
            keep_live(dd, g)
            continue
        rcp = alg.tile([88, 352], F32, tag="rcp")
        nc.vector.reciprocal(rcp[:], dd[:])
        if DBG_STAGE == 7:
            keep_live(nn, g)
            keep_live(rcp, g)
            continue
        m_t = alg.tile([88, 352], F32, tag="m")
        nc.vector.tensor_mul(m_t[:], nn[:], rcp[:])
        mj = alg.tile([88, 352], F32, tag="mjunk")
        nc.scalar.activation(
            mj[:], m_t[:], CPY, accum_out=ssim_stat[:88, g : g + 1]
        )

    # write stats out
    nc.sync.dma_start(l1_out[:], l1_stat[:])
    nc.sync.dma_start(ssim_out[:], ssim_stat[:])


_CACHED = {}


def _get_built(n_groups=N_GROUPS):
    if n_groups not in _CACHED:
        _CACHED[n_groups] = build_kernel(n_groups)
    return _CACHED[n_groups]


def run_cores(predicted: np.ndarray, target: np.ndarray, **run_kwargs):
    predicted = np.asarray(predicted, dtype=np.float32)
    target = np.asarray(target, dtype=np.float32)
    nc = _get_built()
    m4t, w4, w4h, w4hn = make_consts()
    xs = predicted.reshape(N_CORES, CHIMGS_PER_CORE, HW * HW)
    ys = target.reshape(N_CORES, CHIMGS_PER_CORE, HW * HW)
    in_maps = [
        {"x": xs[i], "y": ys[i], "m4t": m4t, "w4": w4, "w4h": w4h,
         "w4hn": w4hn}
        for i in range(N_CORES)
    ]
    res = run_bass_kernel_spmd(
        nc, in_maps, core_ids=list(range(N_CORES)), **run_kwargs
    )
    l1_sum = 0.0
    ssim_sum = 0.0
    for i in range(N_CORES):
        l1_sum += float(res.results[i]["l1stat"].astype(np.float64).sum())
        ssim_sum += float(res.results[i]["ssimstat"].astype(np.float64).sum())
    n_px = float(BATCH * CH * HW * HW)
    n_out = float(BATCH * CH * OUT * OUT)
    l1 = l1_sum / n_px
    ssim = ssim_sum / n_out
    loss = l1 + SSIM_WEIGHT * (1.0 - ssim)
    return res, np.float32(loss)


def kernel(predicted: np.ndarray, target: np.ndarray) -> np.ndarray:
    _, loss = run_cores(predicted, target)
    return loss


# revision 27
# speedup vs baseline: 108.7814x; 108.7814x over previous
"""Trainium2 Bass kernel for L1 + SSIM diffusion loss.

loss = mean|x-y| + 0.1 * (1 - mean(ssim_map(x, y)))

Data-parallel over 8 NeuronCores: each core processes 1024 images
(3072 channel-images of 32x32). Per core the SSIM separable gaussian
blurs are computed on the tensor engine as banded matmuls:
  H-blur:  out1 = M4^T @ tile      (block-diag banded lhsT, 4 row-blocks)
  32x32 block transpose on DVE
  W-blur:  out3 = W4^T @ out1^T'   (block-diag banded lhsT)
The SSIM algebra runs in the S=x+y / D=x-y basis:
  P = B(S) = mu1+mu2, Q = B(D) = mu1-mu2
  2*mu1*mu2   = (P^2-Q^2)/2        mu1^2+mu2^2 = (P^2+Q^2)/2
  2*sigma12   = (B(S^2)-B(D^2))/2 - (P^2-Q^2)/2
  sig1+sig2   = (B(S^2)+B(D^2))/2 - (P^2+Q^2)/2
Per-core partial sums (sum|D| and sum ssim_map) are returned as
[128, n_groups] stat tiles and combined on the host.
"""

import os
import sys

sys.path.insert(0, "/opt/trn_rl_repo")

import math
from contextlib import ExitStack

DBG_STAGE = int(os.environ.get("K_STAGE", "8"))

import numpy as np

import concourse.bass as bass
import concourse.tile as tile
from concourse import bacc, mybir
from concourse.bass_utils import run_bass_kernel_spmd

F32 = mybir.dt.float32

N_CORES = 8
BATCH = 8192
CH = 3
HW = 32
WIN = 11
OUT = HW - WIN + 1  # 22
SIGMA = 1.5
DATA_RANGE = 1.0
K1, K2 = 0.01, 0.03
C1 = (K1 * DATA_RANGE) ** 2
C2 = (K2 * DATA_RANGE) ** 2
SSIM_WEIGHT = 0.1

CHIMGS_PER_CORE = BATCH // N_CORES * CH  # 3072
GROUP = 64  # channel-images per group
N_GROUPS = CHIMGS_PER_CORE // GROUP  # 48


def _gaussian_1d():
    coords = np.arange(WIN, dtype=np.float64) - (WIN - 1) / 2.0
    g = np.exp(-(coords**2) / (2.0 * SIGMA**2))
    g = g / g.sum()
    return g.astype(np.float32)


def _blur_mats():
    """M: [OUT, HW] h-blur matrix (out = M @ img); same matrix for w-blur."""
    g = _gaussian_1d()
    M = np.zeros((OUT, HW), dtype=np.float32)
    for i in range(OUT):
        M[i, i : i + WIN] = g
    return M


def make_consts():
    M = _blur_mats()
    # H-blur lhsT: [128, 128] block-diag of 4x M^T (padded 22->32 out rows)
    m4t = np.zeros((128, 128), dtype=np.float32)
    for b in range(4):
        m4t[b * 32 : b * 32 + HW, b * 32 : b * 32 + OUT] = M.T
    # W-blur lhsT: [128, 88] block-diag of 4x M^T (dense 22 out cols)
    w4 = np.zeros((128, 88), dtype=np.float32)
    for b in range(4):
        w4[b * 32 : b * 32 + HW, b * 22 : b * 22 + OUT] = M.T
    w4h = (0.5 * w4).astype(np.float32)
    w4hn = (-0.5 * w4).astype(np.float32)
    return m4t, w4, w4h, w4hn


def build_kernel(n_groups=N_GROUPS, bench_reps=1):
    nc = bacc.Bacc(
        "TRN2", target_bir_lowering=False, debug=False, num_devices=N_CORES
    )
    n_chimgs = n_groups * GROUP
    x_ap = nc.dram_tensor(
        "x", [n_chimgs, HW * HW], F32, kind="ExternalInput"
    ).ap()
    y_ap = nc.dram_tensor(
        "y", [n_chimgs, HW * HW], F32, kind="ExternalInput"
    ).ap()
    m4t_ap = nc.dram_tensor("m4t", [128, 128], F32, kind="ExternalInput").ap()
    w4_ap = nc.dram_tensor("w4", [128, 88], F32, kind="ExternalInput").ap()
    w4h_ap = nc.dram_tensor("w4h", [128, 88], F32, kind="ExternalInput").ap()
    w4hn_ap = nc.dram_tensor("w4hn", [128, 88], F32, kind="ExternalInput").ap()
    l1_out = nc.dram_tensor(
        "l1stat", [128, n_groups], F32, kind="ExternalOutput"
    ).ap()
    ssim_out = nc.dram_tensor(
        "ssimstat", [128, n_groups], F32, kind="ExternalOutput"
    ).ap()

    with tile.TileContext(nc) as tc:
        with ExitStack() as ctx:
            if bench_reps > 1:
                with tc.For_i(0, bench_reps, 1):
                    kernel_body(
                        ctx, tc, x_ap, y_ap, m4t_ap, w4_ap, w4h_ap, w4hn_ap,
                        l1_out, ssim_out, n_groups,
                    )
            else:
                kernel_body(
                    ctx, tc, x_ap, y_ap, m4t_ap, w4_ap, w4h_ap, w4hn_ap,
                    l1_out, ssim_out, n_groups,
                )
    nc.compile()
    return nc


def kernel_body(ctx, tc, x_ap, y_ap, m4t_ap, w4_ap, w4h_ap, w4hn_ap,
                l1_out, ssim_out, n_groups):
    nc = tc.nc
    sub = mybir.AluOpType.subtract
    add = mybir.AluOpType.add
    mult = mybir.AluOpType.mult
    SQ = mybir.ActivationFunctionType.Square
    ABS = mybir.ActivationFunctionType.Abs
    CPY = mybir.ActivationFunctionType.Copy

    consts = ctx.enter_context(tc.tile_pool(name="consts", bufs=1))
    inp = ctx.enter_context(tc.tile_pool(name="inp", bufs=3))
    maps = ctx.enter_context(tc.tile_pool(name="maps", bufs=2))
    psum1 = ctx.enter_context(tc.tile_pool(name="psum1", bufs=3, space="PSUM"))
    psum3 = ctx.enter_context(tc.tile_pool(name="psum3", bufs=1, space="PSUM"))
    tts = ctx.enter_context(tc.tile_pool(name="tts", bufs=2))
    alg = ctx.enter_context(tc.tile_pool(name="alg", bufs=2))
    stats = ctx.enter_context(tc.tile_pool(name="stats", bufs=1))

    m4t = consts.tile([128, 128], F32)
    nc.sync.dma_start(m4t[:], m4t_ap[:])
    w4 = consts.tile([128, 88], F32)
    nc.sync.dma_start(w4[:], w4_ap[:])
    w4h = consts.tile([128, 88], F32)
    nc.sync.dma_start(w4h[:], w4h_ap[:])
    w4hn = consts.tile([128, 88], F32)
    nc.sync.dma_start(w4hn[:], w4hn_ap[:])

    l1_stat = stats.tile([128, n_groups], F32, tag="l1stat")
    ssim_stat = stats.tile([128, n_groups], F32, tag="ssimstat")
    nc.vector.memset(l1_stat[:], 0.0)
    nc.vector.memset(ssim_stat[:], 0.0)
    c1b = consts.tile([128, 1], F32, tag="c1b")
    nc.vector.memset(c1b[:], C1)
    c2b = consts.tile([128, 1], F32, tag="c2b")
    nc.vector.memset(c2b[:], C2)

    for g in range(n_groups):
        # ---- load: [128, 512] = (b,k) x (q,j); chimg c = g*64 + b*16 + q
        x_t = inp.tile([128, 512], F32, tag="x")
        y_t = inp.tile([128, 512], F32, tag="y")
        for t, ap in ((x_t, x_ap), (y_t, y_ap)):
            for b in range(4):
                src = ap[
                    g * GROUP + b * 16 : g * GROUP + (b + 1) * 16, :
                ].rearrange("q (k j) -> k q j", k=HW, j=HW)
                dst = t[b * 32 : (b + 1) * 32, :].rearrange(
                    "k (q j) -> k q j", q=16, j=HW
                )
                nc.sync.dma_start(dst, src)

        def keep_live(t, col):
            j = maps.tile(list(t.shape), F32, tag="keeplive")
            nc.scalar.activation(
                j[: t.shape[0]], t[:], ABS,
                accum_out=ssim_stat[: t.shape[0], col : col + 1],
            )

        if DBG_STAGE == 1:
            keep_live(x_t, g)
            keep_live(y_t, g)
            continue

        # ---- S, D, S^2, D^2, |D| accumulation
        s_t = maps.tile([128, 512], F32, tag="S")
        nc.vector.tensor_add(s_t[:], x_t[:], y_t[:])
        d_t = maps.tile([128, 512], F32, tag="D")
        nc.vector.tensor_sub(d_t[:], x_t[:], y_t[:])
        s2_t = maps.tile([128, 512], F32, tag="S2")
        nc.scalar.activation(s2_t[:], s_t[:], SQ)
        d2_t = maps.tile([128, 512], F32, tag="D2")
        abs_junk = maps.tile([128, 512], F32, tag="absjunk")
        nc.scalar.activation(
            abs_junk[:], d_t[:], ABS, accum_out=l1_stat[:, g : g + 1]
        )
        nc.scalar.activation(d2_t[:], d_t[:], SQ)

        if DBG_STAGE == 2:
            keep_live(s2_t, g)
            keep_live(d2_t, g)
            continue

        # ---- H-blur + transpose per map
        tts_of = {}
        for name, src_t in (
            ("S", s_t), ("D", d_t), ("S2", s2_t), ("D2", d2_t),
        ):
            o1 = psum1.tile([128, 512], F32, tag="out1")
            nc.tensor.matmul(o1[:], m4t[:], src_t[:], start=True, stop=True)
            o1s = tts.tile([128, 512], F32, tag="o1s" + name)
            nc.scalar.copy(o1s[:], o1[:])
            tt = tts.tile([128, 512], F32, tag="tt" + name)
            nc.vector.transpose(tt[:], o1s[:])
            tts_of[name] = tt

        if DBG_STAGE == 3:
            for tt in tts_of.values():
                keep_live(tt, g)
            continue

        def wrhs(tt):
            return tt[:].rearrange("p (q i) -> p q i", q=16, i=HW)[:, :, 0:OUT]

        # ---- W-blur matmuls; G/H formed by PSUM accumulation
        P = psum3.tile([88, 352], F32, tag="out3P")
        nc.tensor.matmul(P[:], w4[:], wrhs(tts_of["S"]), start=True, stop=True)
        Q = psum3.tile([88, 352], F32, tag="out3Q")
        nc.tensor.matmul(Q[:], w4[:], wrhs(tts_of["D"]), start=True, stop=True)
        G = psum3.tile([88, 352], F32, tag="out3G")  # 2 B(xy)
        nc.tensor.matmul(G[:], w4h[:], wrhs(tts_of["S2"]), start=True, stop=False)
        nc.tensor.matmul(G[:], w4hn[:], wrhs(tts_of["D2"]), start=False, stop=True)
        H = psum3.tile([88, 352], F32, tag="out3H")  # B(x^2) + B(y^2)
        nc.tensor.matmul(H[:], w4h[:], wrhs(tts_of["S2"]), start=True, stop=False)
        nc.tensor.matmul(H[:], w4h[:], wrhs(tts_of["D2"]), start=False, stop=True)

        if DBG_STAGE == 4:
            for t in (P, Q, G, H):
                keep_live(t, g)
            continue

        # ---- ssim algebra on [88, 352]
        rt = math.sqrt(0.5)
        U = alg.tile([88, 352], F32, tag="U")
        nc.scalar.activation(U[:], P[:], SQ, scale=rt)  # 0.5*P^2
        V = alg.tile([88, 352], F32, tag="V")
        nc.scalar.activation(V[:], Q[:], SQ, scale=rt)  # 0.5*Q^2
        A1 = alg.tile([88, 352], F32, tag="A1")
        nc.vector.tensor_sub(A1[:], U[:], V[:])  # 2 mu1 mu2
        A2 = alg.tile([88, 352], F32, tag="A2")
        nc.vector.tensor_add(A2[:], U[:], V[:])  # mu1^2 + mu2^2
        num1 = alg.tile([88, 352], F32, tag="num1")
        nc.scalar.add(num1[:], A1[:], c1b[:88, :])
        den1 = alg.tile([88, 352], F32, tag="den1")
        nc.scalar.add(den1[:], A2[:], c1b[:88, :])
        tn = alg.tile([88, 352], F32, tag="tn")
        nc.vector.tensor_sub(tn[:], G[:], A1[:])  # 2 sigma12
        num2 = alg.tile([88, 352], F32, tag="num2")
        nc.scalar.add(num2[:], tn[:], c2b[:88, :])
        td = alg.tile([88, 352], F32, tag="td")
        nc.vector.tensor_sub(td[:], H[:], A2[:])  # sig1^2 + sig2^2
        den2 = alg.tile([88, 352], F32, tag="den2")
        nc.scalar.add(den2[:], td[:], c2b[:88, :])
        if DBG_STAGE == 5:
            for t in (num1, num2, den1, den2):
                keep_live(t, g)
            continue
        nn = alg.tile([88, 352], F32, tag="nn")
        nc.vector.tensor_mul(nn[:], num1[:], num2[:])
        dd = alg.tile([88, 352], F32, tag="dd")
        nc.vector.tensor_mul(dd[:], den1[:], den2[:])
        if DBG_STAGE == 6:
            keep_live(nn, g)
            keep_live(dd, g)
            continue
        rcp = alg.tile([88, 352], F32, tag="rcp")
        nc.vector.reciprocal(rcp[:], dd[:])
        if DBG_STAGE == 7:
            keep_live(nn, g)
            keep_live(rcp, g)
            continue
        m_t = alg.tile([88, 352], F32, tag="m")
        nc.vector.tensor_mul(m_t[:], nn[:], rcp[:])
        mj = alg.tile([88, 352], F32, tag="mjunk")
        nc.scalar.activation(
            mj[:], m_t[:], CPY, accum_out=ssim_stat[:88, g : g + 1]
        )

    # write stats out
    nc.sync.dma_start(l1_out[:], l1_stat[:])
    nc.sync.dma_start(ssim_out[:], ssim_stat[:])


_CACHED = {}


def _get_built(n_groups=N_GROUPS):
    if n_groups not in _CACHED:
        _CACHED[n_groups] = build_kernel(n_groups)
    return _CACHED[n_groups]


def run_cores(predicted: np.ndarray, target: np.ndarray, **run_kwargs):
    predicted = np.asarray(predicted, dtype=np.float32)
    target = np.asarray(target, dtype=np.float32)
    nc = _get_built()
    m4t, w4, w4h, w4hn = make_consts()
    xs = predicted.reshape(N_CORES, CHIMGS_PER_CORE, HW * HW)
    ys = target.reshape(N_CORES, CHIMGS_PER_CORE, HW * HW)
    in_maps = [
        {"x": xs[i], "y": ys[i], "m4t": m4t, "w4": w4, "w4h": w4h,
         "w4hn": w4hn}
        for i in range(N_CORES)
    ]
    res = run_bass_kernel_spmd(
        nc, in_maps, core_ids=list(range(N_CORES)), **run_kwargs
    )
    l1_sum = 0.0
    ssim_sum = 0.0
    for i in range(N_CORES):
        l1_sum += float(res.results[i]["l1stat"].astype(np.float64).sum())
        ssim_sum += float(res.results[i]["ssimstat"].astype(np.float64).sum())
    n_px = float(BATCH * CH * HW * HW)
    n_out = float(BATCH * CH * OUT * OUT)
    l1 = l1_sum / n_px
    ssim = ssim_sum / n_out
    loss = l1 + SSIM_WEIGHT * (1.0 - ssim)
    return res, np.float32(loss)


def kernel(predicted: np.ndarray, target: np.ndarray) -> np.ndarray:
    _, loss = run_cores(predicted, target)
    return loss
